# revision 46
# baseline (speedup 1.0000x reference)
"""Multi-head attention block (B=4, S=2048, D=1024, H=16) on 8 TRN2 NeuronCores.

Sharding: core c handles batch b = c//2 and head-group hg = c%2 (8 heads,
a 512-wide slice of the qkv projections). No collectives: each core
computes a [D, S] transposed partial of the output projection for its
head group; the host sums the two head-group partials per batch, adds
the output bias, and transposes back to [S, D].

Per-core dataflow (bf16 compute, f32 PSUM accumulation):
  - host pre-casts all big inputs to bf16 AND pre-transposes q/k/v to
    [D, S] (so the device does no casting and no transposing)
  - Q^T/K^T from projections (dout on partitions); biases are folded
    into the PSUM->SBUF evictions as DVE tensor_scalar adds (per-
    partition f32 bias operand) -- no bias matmuls on the PE
  - V in natural [s, dout] layout with bias added via a broadcast
    tensor_tensor at eviction; augmented with a ones column per head
    (softmax denominators ride along the attn@V matmul as a 65th row)
  - scores^T [k, q] per head via zero-padded K=128 matmuls (uniform
    128x128 tile mode); exp on ACT (PSUM -> SBUF bf16, scale=1/8);
    O_aug accumulated over k tiles in PSUM; normalization via DVE
    reciprocal + GPSIMD partition-broadcast + DVE multiply
  - out^T = Wo^T O^T -> [D, S] bf16 -> DMA out (host sums partials
    in f32)
  - startup: input DMAs are emitted big-first in consumption order
    with xq chunked so the sc-major q-projections can start as soon
    as the first s-chunk lands; the tail splits the last s-chunk's
    output projection into ht<3 partial accumulations (overlapping
    the final normalize) plus a short ht=3 finish
"""

import numpy as np
import ml_dtypes

import concourse.bass as bass
import concourse.bacc as bacc
import concourse.mybir as mybir
from concourse.tile import TileContext
from concourse.bass import ds

F32 = mybir.dt.float32
BF16 = mybir.dt.bfloat16
EXP = mybir.ActivationFunctionType.Exp

B, S, D, H, HD = 4, 2048, 1024, 16, 64
N_CORES = 8
HPC = H // (N_CORES // B)          # heads per core = 8
DV = HPC * HD                      # 512


def build_attn_core(S=2048, D=1024, HPC=8, HD=64):
    DV = HPC * HD            # head-group width
    NPAIR = HPC // 2         # head pairs; DV = NPAIR * 128
    NDT = D // 128           # din tiles
    NKT = S // 128           # key tiles
    QC = 512                 # q chunk
    NQC = S // QC
    SC = 512                 # s chunk for projections
    NSC = S // SC
    SCALE = HD ** -0.5

    nc = bacc.Bacc("TRN2", target_bir_lowering=False)
    q_ext = nc.dram_tensor("queryT", [D, S], BF16, kind="ExternalInput")
    k_ext = nc.dram_tensor("keyT", [D, S], BF16, kind="ExternalInput")
    v_ext = nc.dram_tensor("valueT", [D, S], BF16, kind="ExternalInput")
    wq_ext = nc.dram_tensor("Wq", [D, DV], BF16, kind="ExternalInput")
    wk_ext = nc.dram_tensor("Wk", [D, DV], BF16, kind="ExternalInput")
    wv_ext = nc.dram_tensor("Wv", [D, DV], BF16, kind="ExternalInput")
    wo_ext = nc.dram_tensor("Wo", [DV, D], BF16, kind="ExternalInput")
    bq_ext = nc.dram_tensor("bq", [DV], F32, kind="ExternalInput")
    bk_ext = nc.dram_tensor("bk", [DV], F32, kind="ExternalInput")
    bv_ext = nc.dram_tensor("bv", [DV], F32, kind="ExternalInput")
    out_ext = nc.dram_tensor("out", [D, S], BF16, kind="ExternalOutput")

    with TileContext(nc) as tc:
        with (
            tc.tile_pool(name="const", bufs=1) as cpool,
            tc.tile_pool(name="big", bufs=1) as big,
            tc.tile_pool(name="pt", bufs=8) as ptpool,
            tc.tile_pool(name="vl", bufs=3) as vlpool,
            tc.tile_pool(name="rec", bufs=2) as recpool,
            tc.tile_pool(name="recb", bufs=2) as recbpool,
            tc.tile_pool(name="oun", bufs=4) as ounpool,
            tc.tile_pool(name="stage", bufs=3) as stage,
            tc.tile_pool(name="mmps", bufs=2, space="PSUM") as mmps,
            tc.tile_pool(name="scps", bufs=2, space="PSUM") as scps,
            tc.tile_pool(name="ops", bufs=2, space="PSUM") as opool,
        ):
            # -------- SBUF claims: big pool FIRST so every hot tensor
            # lands at a 128B-aligned SBUF base (a const-pool-first layout
            # put all matmul operands at offset 64 mod 128, costing ~20%
            # streaming rate on PE and ACT). Claim order here is SBUF
            # allocation order; DMA emission order is set separately below.
            xqT = big.tile([128, NDT, S], BF16, tag="xqT")
            xkT = big.tile([128, NDT, S], BF16, tag="xkT")
            wq_sb = big.tile([128, NDT, DV], BF16, tag="wq")
            wk_sb = big.tile([128, NDT, DV], BF16, tag="wk")
            wv_sb = big.tile([128, NDT, DV], BF16, tag="wv")
            wo_sb = big.tile([128, NPAIR, D], BF16, tag="wo")
            qT = big.tile([128, NPAIR, S], BF16, tag="qT")
            kTe = big.tile([128, NPAIR, S], BF16, tag="kTe")
            kTo = big.tile([128, NPAIR, S], BF16, tag="kTo")
            v_aug = big.tile([128, NKT, HPC * 65], BF16, tag="vaug")
            bqf = cpool.tile([128, NPAIR], F32, tag="bqf")
            bkf = cpool.tile([128, NPAIR], F32, tag="bkf")
            bvrow = cpool.tile([1, DV], F32, tag="bvrow")
            bv_bc = cpool.tile([128, DV], F32, tag="bvbc")
            zpad = cpool.tile([128, SC], BF16, tag="zpad")
            # pad the const pool to a 128B multiple so later pools stay
            # 128B-aligned (claims above total 5152B per partition)
            alignpad = cpool.tile([128, 24], F32, tag="alignpad")  # noqa: F841

            # -------- tiny f32 bias loads FIRST (the PSUM evictions need
            # them, so they must not queue behind the big transfers). bq/bk
            # land as [128, NPAIR] (partition p, head-pair t) so the
            # eviction's tensor_scalar can take a [128, 1] per-partition
            # operand; bv is broadcast to all partitions once for the
            # [s, dout]-layout V eviction.
            nc.sync.dma_start(bqf[:], bq_ext.rearrange("(t p) -> p t", p=128))
            nc.sync.dma_start(bkf[:], bk_ext.rearrange("(t p) -> p t", p=128))
            nc.sync.dma_start(bvrow[:], bv_ext.rearrange("(a n) -> a n", a=1))
            nc.gpsimd.partition_broadcast(bv_bc[:], bvrow[:])
            nc.vector.memset(zpad[:], 0.0)

            # -------- big inputs: emission order = consumption order
            # (q path before k path before v/o). xq is chunked so sc-major
            # q-projections can start after the first s-chunk instead of
            # the whole tensor. First-needed tensors split in half so two
            # DMA engines carry each (per-queue bandwidth limits the ramp).
            wqr = wq_ext.rearrange("(t p) n -> p t n", p=128)
            nc.sync.dma_start(wq_sb[:, 0:4, :], wqr[:, 0:4, :])
            nc.sync.dma_start(wq_sb[:, 4:8, :], wqr[:, 4:8, :])
            xq0 = q_ext[:, ds(0, SC)].rearrange("(t p) s -> p t s", p=128)
            nc.sync.dma_start(xqT[:, 0:4, ds(0, SC)], xq0[:, 0:4, :])
            nc.sync.dma_start(xqT[:, 4:8, ds(0, SC)], xq0[:, 4:8, :])
            nc.sync.dma_start(wk_sb[:], wk_ext.rearrange("(t p) n -> p t n", p=128))
            for c in range(1, NSC):
                xqc = q_ext[:, ds(c * SC, SC)].rearrange("(t p) s -> p t s", p=128)
                nc.sync.dma_start(xqT[:, 0:4, ds(c * SC, SC)], xqc[:, 0:4, :])
                nc.sync.dma_start(xqT[:, 4:8, ds(c * SC, SC)], xqc[:, 4:8, :])
            nc.sync.dma_start(
                xkT[:, :, ds(0, 1024)],
                k_ext[:, ds(0, 1024)].rearrange("(t p) s -> p t s", p=128),
            )
            # xk's second half, wv and wo are gated behind a DVE marker
            # emitted after the first q-projection group, so they don't
            # steal HBM bandwidth from the q path during the ramp
            def late_dmas():
                nc.sync.dma_start(
                    xkT[:, :, ds(1024, 1024)],
                    k_ext[:, ds(1024, 1024)].rearrange("(t p) s -> p t s", p=128),
                )
                nc.sync.dma_start(wv_sb[:], wv_ext.rearrange("(t p) n -> p t n", p=128))
                nc.sync.dma_start(wo_sb[:], wo_ext.rearrange("(t p) n -> p t n", p=128))

            # -------- persistent SBUF aliases ----------------------------
            # kT is stored twice with complementary halves zeroed, so the
            # scores matmuls can use full K=128 operands (uniform 128x128
            # tile mode, no mode-switch drains): the zero rows of the
            # stationary operand nullify the other head's contribution.
            oT = big.tile([128, NPAIR, S], BF16, tag="xqT")  # reuse xqT memory (dead after Q-projs)
            # bf16 staging for the last s-chunk's ht<3 outproj partials
            # (accumulated mid-chunk as fillers; PSUM banks freed right
            # away so the tail is just 8 single ht=3 matmuls + adds).
            # Reuses xkT memory: dead once k_proj(t=3, sc=3) has run
            # (inside chunk (t=2, qc=3)), well before these are written.
            opart = big.tile([128, NDT, S], BF16, tag="xkT")
            nc.gpsimd.memset(kTe[64:128, :, :], 0.0)
            nc.gpsimd.memset(kTo[0:64, :, :], 0.0)

            # -------- work-chunk emitters --------------------------------
            def q_proj_sc(t, sc, pool=None):
                pool, tag = pool or (mmps, "mm")
                ps = pool.tile([128, SC], F32, tag=tag)
                for dk in range(NDT):
                    nc.tensor.matmul(
                        ps[:],
                        wq_sb[:, dk, ds(t * 128, 128)],
                        xqT[:, dk, ds(sc * SC, SC)],
                        start=(dk == 0),
                        stop=(dk == NDT - 1),
                    )
                nc.vector.tensor_scalar_add(
                    qT[:, t, ds(sc * SC, SC)], ps[:], bqf[:, ds(t, 1)]
                )

            def k_proj_sc(t, sc, pool=None):
                pool, tag = pool or (mmps, "mm")
                ps = pool.tile([128, SC], F32, tag=tag)
                for dk in range(NDT):
                    nc.tensor.matmul(
                        ps[:],
                        wk_sb[:, dk, ds(t * 128, 128)],
                        xkT[:, dk, ds(sc * SC, SC)],
                        start=(dk == 0),
                        stop=(dk == NDT - 1),
                    )
                nc.vector.tensor_scalar_add(
                    kTe[0:64, t, ds(sc * SC, SC)], ps[0:64, :], bkf[0:64, ds(t, 1)]
                )
                nc.vector.tensor_scalar_add(
                    kTo[64:128, t, ds(sc * SC, SC)], ps[64:128, :], bkf[64:128, ds(t, 1)]
                )

            def v_proj_st(st):
                vl = vlpool.tile([128, NDT, 128], BF16, tag="vl")
                nc.sync.dma_start(
                    vl[:],
                    v_ext[:, ds(st * 128, 128)].rearrange("(t p) s -> p t s", p=128),
                )
                ps = mmps.tile([128, DV], F32, tag="mm")
                for dk in range(NDT):
                    nc.tensor.matmul(
                        ps[:],
                        vl[:, dk, :],
                        wv_sb[:, dk, :],
                        start=(dk == 0),
                        stop=(dk == NDT - 1),
                    )
                dst = v_aug[:, st, :].rearrange("p (h c) -> p h c", c=65)
                nc.vector.tensor_add(
                    dst[:, :, 0:64],
                    ps[:].rearrange("p (h c) -> p h c", c=64),
                    bv_bc[:].rearrange("p (h c) -> p h c", c=64),
                )
                nc.vector.memset(dst[:, :, 64:65], 1.0)

            def outproj_dt(sc, dt2):
                po = mmps.tile([128, SC], F32, tag="mm")
                for ht in range(NPAIR):
                    nc.tensor.matmul(
                        po[:],
                        wo_sb[:, ht, ds(dt2 * 128, 128)],
                        oT[:, ht, ds(sc * SC, SC)],
                        start=(ht == 0),
                        stop=(ht == NPAIR - 1),
                    )
                ost = stage.tile([128, SC], BF16, tag="ostage")
                nc.vector.tensor_copy(ost[:], po[:])
                nc.sync.dma_start(
                    out_ext[ds(dt2 * 128, 128), ds(sc * SC, SC)], ost[:]
                )

            def outproj_partial_dt(sc, dt2):
                # ht 0..2 of the last s-chunk, staged to SBUF bf16 so the
                # mmps bank frees immediately
                po = mmps.tile([128, SC], F32, tag="mm")
                for ht in range(NPAIR - 1):
                    nc.tensor.matmul(
                        po[:],
                        wo_sb[:, ht, ds(dt2 * 128, 128)],
                        oT[:, ht, ds(sc * SC, SC)],
                        start=(ht == 0),
                        stop=(ht == NPAIR - 2),
                    )
                nc.vector.tensor_copy(opart[:, dt2, 0:SC], po[:])

            # -------- HAM warm-up ----------------------------------------
            # ~5us of dependency-free matmuls on the zeros tile: keeps the
            # PE busy during the initial DMA-only window so the activity
            # monitor never throttles the clock before the projections.
            for wi in range(14):
                wps = mmps.tile([128, SC], F32, tag="mm")
                for _ in range(2):
                    nc.tensor.matmul(
                        wps[:], zpad[:, 0:128], zpad[:],
                        start=True, stop=True,
                    )

            # -------- projections needed before attention(0) -------------
            # sc-major so chunk i only needs xq's s-chunk i (matches the
            # chunked DMA arrival order). Rotate PSUM groups through the
            # (still idle) attention pools so the PE pipeline stays dense.
            rot = [(mmps, "mm"), (scps, "sc"), (opool, "o")]
            ri = 0
            for sc in range(NSC):
                for t in range(NPAIR):
                    q_proj_sc(t, sc, pool=rot[ri % 3]); ri += 1
                if sc == 0:
                    # WAR markers: the late DMAs overwrite these bytes, so
                    # they wait for the DVE to reach this point (i.e. the
                    # first q-projection group is done) before starting
                    nc.vector.memset(xkT[:, 0, 1024:1025], 0.0)
                    nc.vector.memset(wv_sb[:, 0, 0:1], 0.0)
                    nc.vector.memset(wo_sb[:, 0, 0:1], 0.0)
                    late_dmas()
            for sc in range(NSC):
                k_proj_sc(0, sc, pool=rot[ri % 3]); ri += 1

            # -------- attention ------------------------------------------
            # ACT (exp) is the long pole; every other PE-work chunk is
            # interleaved into the kt loop as "filler" so the in-order PE
            # queue never parks a long burst in front of the next scores.
            SCALEF = SCALE

            deferred = []

            def normalize_one(o_sb, t, qc, hh, hop_on_dve=False):
                # Move the denominator row to partition 0 with a tiny
                # SBUF->SBUF DMA (partition-free, idle engines), broadcast,
                # then take the reciprocal on all 64 lanes in parallel --
                # a [1, N] reciprocal runs serially on one DVE lane (3.3us)
                # and the HW partition-broadcast only reads partition 0.
                # In the latency-critical tail the DMA hop (~2-4us queue
                # latency) is replaced by an on-engine DVE copy (~0.3us).
                dn = recpool.tile([64, QC], F32, tag="rec")
                if hop_on_dve:
                    nc.vector.tensor_copy(dn[0:1, :], o_sb[64:65, :])
                else:
                    nc.sync.dma_start(dn[0:1, :], o_sb[64:65, :])
                bc = recbpool.tile([64, QC], F32, tag="recb")
                nc.gpsimd.partition_broadcast(bc[:], dn[0:1, :])
                # ~51 ULP, ~5x faster than reciprocal() (which costs ~3.3us
                # per 512 columns regardless of partition count)
                nc.vector.reciprocal_approx_fast(out=bc[:], in_=bc[:])
                nc.vector.tensor_mul(
                    oT[ds(hh * 64, 64), t, ds(qc * QC, QC)],
                    o_sb[0:64, :],
                    bc[:],
                )

            def attention_pair(t):
                for qc in range(NQC):
                    last = t == NPAIR - 1 and qc == NQC - 1
                    # filler thunks interleaved after each scores/exp step;
                    # lag = how many k-tiles attn@V trails the exp stream
                    # (deep for the very first chunk so attn@V can wait for
                    # the V projection without stalling the exp feed)
                    fillers = {}
                    lag = 4
                    if t == 0 and qc == 0:
                        # V projection interleaved 2 tiles per kt from kt=1;
                        # attn@V trails by 4 k-tiles so v_aug[kt] is always
                        # emitted (and computed) before its consumer
                        for st in range(NKT):
                            fillers.setdefault(min(1 + st, NKT - 1), []).append(
                                lambda st=st: v_proj_st(st)
                            )
                    if t + 1 < NPAIR and qc == NQC - 1:
                        for i in range(NSC):
                            fillers.setdefault(2 + 4 * i, []).append(
                                lambda tt=t + 1, sc=i: k_proj_sc(tt, sc)
                            )
                    if t == NPAIR - 1 and qc > 0:
                        # kt >= 6 only: after the deferred normalizations of
                        # the previous chunk (popped at kt 2 and 4) have
                        # written the oT slices these consume
                        for i in range(NDT):
                            fillers.setdefault(5 + i, []).append(
                                lambda sc=qc - 1, dt2=i: outproj_dt(sc, dt2)
                            )
                    if last:
                        # the last s-chunk's ht<3 partials only read
                        # oT[:, 0:3, sc3], all normalized chunks ago
                        for i in range(NDT):
                            fillers.setdefault(6 + i, []).append(
                                lambda dt2=i: outproj_partial_dt(qc, dt2)
                            )

                    oA = opool.tile([65, QC], F32, tag="o")
                    oB = opool.tile([65, QC], F32, tag="o")
                    pts = {}

                    def scores_exp(kt):
                        sct = scps.tile([128, 2 * QC], F32, tag="sc")
                        nc.tensor.matmul(
                            sct[:, 0:QC],
                            kTe[:, t, ds(kt * 128, 128)],
                            qT[:, t, ds(qc * QC, QC)],
                            start=True, stop=True,
                        )
                        nc.tensor.matmul(
                            sct[:, QC : 2 * QC],
                            kTo[:, t, ds(kt * 128, 128)],
                            qT[:, t, ds(qc * QC, QC)],
                            start=True, stop=True,
                        )
                        pt = ptpool.tile([128, 2 * QC], BF16, tag="pt")
                        nc.scalar.activation(pt[:], sct[:], EXP, bias=0.0, scale=SCALEF)
                        pts[kt] = pt

                    def attn_v(kt):
                        pt = pts.pop(kt)
                        nc.tensor.matmul(
                            oA[:],
                            v_aug[:, kt, ds((2 * t) * 65, 65)],
                            pt[:, 0:QC],
                            start=(kt == 0),
                            stop=(kt == NKT - 1),
                        )
                        nc.tensor.matmul(
                            oB[:],
                            v_aug[:, kt, ds((2 * t + 1) * 65, 65)],
                            pt[:, QC : 2 * QC],
                            start=(kt == 0),
                            stop=(kt == NKT - 1),
                        )

                    scores_exp(0)
                    for f in fillers.get(0, ()):
                        f()
                    for kt in range(1, NKT):
                        scores_exp(kt)
                        for f in fillers.get(kt, ()):
                            f()
                        if kt in (1, 3) and deferred:
                            normalize_one(*deferred.pop(0))
                        if kt - lag >= 0:
                            attn_v(kt - lag)
                    for kt in range(max(0, NKT - lag), NKT):
                        attn_v(kt)

                    # copy both accumulators PSUM->SBUF now (frees the
                    # banks for the next chunk within ~1us); the recip/
                    # broadcast/mul chains are DEFERRED into the next
                    # chunk's loop so they soak up DVE idle time there
                    # instead of stalling this boundary
                    o_sbs = []
                    for o_ps, hh in ((oA, 0), (oB, 1)):
                        o_sb = ounpool.tile([65, QC], F32, tag="oun")
                        nc.vector.tensor_copy(o_sb[:], o_ps[:])
                        if last:
                            o_sbs.append((o_sb, hh))
                        else:
                            deferred.append((o_sb, t, qc, hh))

                    if last:
                        # Tail: the ht<3 partials already ran as fillers.
                        # Two-phase normalize so the lanes' cross-engine
                        # chains overlap (DVE: dn0,dn1 | GPSIMD: bc0,bc1),
                        # dummy matmuls keep the PE (and the activity
                        # monitor's clock) busy while the chain crosses
                        # engines, then 2 finals per scps pair-tile.
                        work = []
                        for o_sb, hh in o_sbs:
                            dn = recpool.tile([64, QC], F32, tag="rec")
                            nc.vector.tensor_copy(dn[0:1, :], o_sb[64:65, :])
                            bc = recbpool.tile([64, QC], F32, tag="recb")
                            nc.gpsimd.partition_broadcast(bc[:], dn[0:1, :])
                            work.append((o_sb, bc, hh))

                        def dummy_mm(n):
                            for _ in range(n):
                                dps = opool.tile([65, QC], F32, tag="o")
                                nc.tensor.matmul(
                                    dps[:], zpad[:, 0:65], zpad[:],
                                    start=True, stop=True,
                                )

                        dummy_mm(16)
                        for o_sb, bc, hh in work:
                            nc.vector.reciprocal_approx_fast(out=bc[:], in_=bc[:])
                            nc.vector.tensor_mul(
                                oT[ds(hh * 64, 64), t, ds(qc * QC, QC)],
                                o_sb[0:64, :],
                                bc[:],
                            )
                        for g in range(NDT // 2):
                            po2 = scps.tile([128, 2 * QC], F32, tag="sc")
                            for j in range(2):
                                dt2 = 2 * g + j
                                nc.tensor.matmul(
                                    po2[:, ds(j * QC, QC)],
                                    wo_sb[:, NPAIR - 1, ds(dt2 * 128, 128)],
                                    oT[:, NPAIR - 1, ds(qc * QC, QC)],
                                    start=True, stop=True,
                                )
                            for j in range(2):
                                dt2 = 2 * g + j
                                ost = stage.tile([128, SC], BF16, tag="ostage")
                                nc.vector.tensor_add(
                                    ost[:], po2[:, ds(j * QC, QC)],
                                    opart[:, dt2, 0:SC],
                                )
                                nc.sync.dma_start(
                                    out_ext[ds(dt2 * 128, 128), ds(qc * QC, QC)],
                                    ost[:],
                                )
                            if g == 1:
                                dummy_mm(4)

            for t in range(NPAIR):
                attention_pair(t)
            while deferred:
                normalize_one(*deferred.pop(0))

    nc.finalize()
    return nc


_NC_CACHE = {}


def _get_nc():
    if "nc" not in _NC_CACHE:
        _NC_CACHE["nc"] = build_attn_core(S=S, D=D, HPC=HPC, HD=HD)
    return _NC_CACHE["nc"]


def _make_in_maps(query, key, value, Wq, bq, Wk, bk, Wv, bv, Wo):
    bf = ml_dtypes.bfloat16
    in_maps = []
    for c in range(N_CORES):
        b, hg = c // 2, c % 2
        sl = slice(hg * DV, (hg + 1) * DV)
        in_maps.append(dict(
            queryT=np.ascontiguousarray(query[b].T).astype(bf),
            keyT=np.ascontiguousarray(key[b].T).astype(bf),
            valueT=np.ascontiguousarray(value[b].T).astype(bf),
            Wq=np.ascontiguousarray(Wq[:, sl]).astype(bf),
            Wk=np.ascontiguousarray(Wk[:, sl]).astype(bf),
            Wv=np.ascontiguousarray(Wv[:, sl]).astype(bf),
            Wo=np.ascontiguousarray(Wo[sl, :]).astype(bf),
            bq=np.ascontiguousarray(bq[sl]).astype(np.float32),
            bk=np.ascontiguousarray(bk[sl]).astype(np.float32),
            bv=np.ascontiguousarray(bv[sl]).astype(np.float32),
        ))
    return in_maps


def _assemble(results, bo):
    out = np.empty((B, S, D), dtype=np.float32)
    for b in range(B):
        part = (results[2 * b]["out"].astype(np.float32)
                + results[2 * b + 1]["out"].astype(np.float32))   # [D, S]
        out[b] = part.T + bo
    return out


def run(inputs, trace=False):
    """Run on 8 cores; returns (output, BassKernelResults)."""
    from concourse.bass_utils import run_bass_kernel_spmd

    inputs = {k: np.asarray(v) for k, v in inputs.items()}
    nc = _get_nc()
    in_maps = _make_in_maps(
        inputs["query"], inputs["key"], inputs["value"],
        inputs["Wq"], inputs["bq"], inputs["Wk"], inputs["bk"],
        inputs["Wv"], inputs["bv"], inputs["Wo"],
    )
    res = run_bass_kernel_spmd(
        nc, in_maps, core_ids=list(range(N_CORES)), trace=trace
    )
    out = _assemble(res.results, np.asarray(inputs["bo"], dtype=np.float32))
    return out, res


def kernel(**inputs) -> np.ndarray:
    out, _ = run(inputs, trace=False)
    return out
